# revision 18
# baseline (speedup 1.0000x reference)
"""Distributed GAT (nn_AdjGAT) kernel for 8 TRN2 NeuronCores — v3 (bf16).

Math: the per-edge softmax logit depends only on the source node, so
    head[h,v] = (sum_k w[h,n_k] * t[h,n_k]) / (sum_k w[h,n_k]),  w = exp(attn)
    out = relu(mean_h(head + b[h]))
Each core computes per-node table rows of 1280 bytes
    [512 bf16 s = w*t | 4 bf16 w | 124 pad]  (row %256B for dma_gather),
AllGathers the table, then per 128-node chunk gathers the 2048 neighbor
rows with one 2048-index dma_gather (Q7 mlp library, 4 SWDGE queues so
transfers overlap) and reduces each node's K=16 rows on the PE with
block-diagonal selection matmuls (numerator and denominator), then
normalizes on DVE and applies relu/mean on ACT.
Phase 1 computes t^T = x^T W directly in node-major layout and
attn = x^T (W a) with host-prefolded aw = W @ a; params are pre-cast to
bf16 on the host so loads go through the HWDGE engines (sync/ACT) and
start immediately (gpsimd is busy with load_library at kernel start).
"""

import math
from contextlib import ExitStack

import ml_dtypes
import numpy as np

import concourse.bass as bass
import concourse.bacc as bacc
import concourse.mybir as mybir
from concourse import library_config
from concourse.bass_utils import run_bass_kernel_spmd

F32 = mybir.dt.float32
BF16 = mybir.dt.bfloat16
I16 = mybir.dt.int16
BF16NP = ml_dtypes.bfloat16

V, D, K, O, H = 20000, 256, 16, 128, 4
NCORES = 8


class Cfg:
    def __init__(self):
        self.V, self.D, self.K, self.O, self.H, self.ncores = V, D, K, O, H, NCORES
        self.VP = V // NCORES            # 2500 nodes per core
        self.ZPAD = 12
        self.VPAD = self.VP + self.ZPAD  # 2512 rows per core shard
        self.VT = self.VPAD * NCORES     # 20096 rows in gathered table
        self.NCH = math.ceil(self.VP / 128)   # 20 chunks of 128 nodes
        self.VPF = self.NCH * 128        # 2560 (padded node count)
        self.DC = D // 128               # 2 contraction tiles
        self.SW = H * O                  # 512 s values per row
        self.RB = 640  # bf16 elems per row: 1280B (%256==0 for dma_gather)
        # row: [0:512) s | [512:516) w | [516:640) pad
        assert self.VT < 32768  # int16 gather indices


def build_graph(cfg: Cfg):
    nc = bacc.Bacc(num_swdge_queues=4, dynamic_dma_scratch_size=32768)
    VP, VPAD, VT, RB, SW, DC, NCH, H_, O_ = (
        cfg.VP, cfg.VPAD, cfg.VT, cfg.RB, cfg.SW, cfg.DC, cfg.NCH, cfg.H, cfg.O)

    xT = nc.declare_dram_parameter("xT", [cfg.D, VP], BF16, isOutput=False)
    W4p = nc.declare_dram_parameter("W4", [cfg.D, SW], BF16, isOutput=False)
    awp = nc.declare_dram_parameter("aw", [cfg.D, H_], BF16, isOutput=False)
    bp = nc.declare_dram_parameter("b", [H_, O_], BF16, isOutput=False)
    blkp = nc.declare_dram_parameter("blk", [128, 248], BF16, isOutput=False)
    idxp = nc.declare_dram_parameter("idx", [128, NCH * 128], I16, isOutput=False)
    out_ext = nc.declare_dram_parameter("out", [VP, O_], F32, isOutput=True)

    tbl_loc = nc.dram_tensor("tbl_loc", [VPAD, RB], BF16)
    tbl = nc.dram_tensor("tbl", [VT, RB], BF16, addr_space="Shared")

    ctx = ExitStack()
    sb = lambda n, s, d: ctx.enter_context(nc.sbuf_tensor(n, s, d))
    xT_sb = sb("xT_sb", [128, DC, cfg.VPF], BF16)
    W_sb = sb("W_sb", [128, DC, SW], BF16)
    aw_sb = sb("aw_sb", [128, DC, H_], BF16)
    b_sb = sb("b_sb", [H_, O_], BF16)
    ones4 = sb("ones4", [H_, O_], BF16)
    blk_sb = sb("blk_sb", [128, 248], BF16)
    idx_sb = sb("idx_sb", [128, NCH * 128], I16)
    binit_sb = sb("binit_sb", [128, O_], F32)
    w4 = [sb(f"w4_{i}", [128, H_], F32) for i in range(2)]
    stage = [sb(f"stage{i}", [128, RB], BF16) for i in range(3)]
    gtile = [sb(f"gtile{i}", [128, 16, RB], BF16) for i in range(6)]
    den_sb = [sb(f"den{i}", [128, H_], F32) for i in range(2)]
    rcp_sb = [sb(f"rcp{i}", [128, H_], F32) for i in range(2)]
    acc_sb = [sb(f"acc{i}", [128, O_], F32) for i in range(2)]
    ostage = [sb(f"ostage{i}", [128, O_], F32) for i in range(2)]
    zero_sb = sb("zero_sb", [cfg.ZPAD, RB], BF16)

    psc = ExitStack()
    ps = lambda n: psc.enter_context(nc.psum_tensor(n, [128, 512], F32))
    ps_t = [ps(f"ps_t{i}") for i in range(2)]
    ps_a = [ps(f"ps_a{i}") for i in range(2)]
    ps_A = [ps(f"ps_A{i}") for i in range(2)]
    ps_den = [ps(f"ps_den{i}") for i in range(2)]
    psc.close()

    sctx = ExitStack()
    sem = lambda n: sctx.enter_context(nc.semaphore(n))
    (s_in, s_in2, s_dvi, s_bini, s_bcp, s_attn, s_w, s_tmm, s_sm,
     s_zr, s_cc, s_mm, s_ep, s_rel) = [sem(n) for n in (
        "s_in", "s_in2", "s_dvi", "s_bini", "s_bcp", "s_attn",
        "s_w", "s_tmm", "s_sm", "s_zr", "s_cc", "s_mm",
        "s_ep", "s_rel")]
    s_sd = [sem("s_sd0"), sem("s_sd1"), sem("s_sd2")]
    s_g = [sem(f"s_g{i}") for i in range(8)]  # per (chunk%8); queue c%4
    # cumulative gather count per queue after chunk c is issued
    qcum = []
    cnt = [0, 0, 0, 0]
    for c in range(NCH):
        for m in range(2):
            cnt[(2 * c + m) % 4] += 1
        qcum.append(tuple(cnt))
    s_o = [sem("s_o0"), sem("s_o1")]
    # stage DMA for group gr incs s_sd[gr%3] to 16*(gr//3+1); a writer of
    # stage[gr%3] needs the buffer's previous DMA (gr-3) done:
    sd_wait = lambda gr: 16 * (gr // 3)

    with nc.Block() as block:
        @block.gpsimd
        def _(g):
            g.load_library(library_config.mlp)
            g.wait_ge(s_dvi, 1)
            g.dma_start(out=bass.AP(tbl_loc, VP * RB, [[RB, cfg.ZPAD], [1, RB]]),
                        in_=zero_sb[:, :]).then_inc(s_zr, 16)
            g.wait_ge(s_zr, 16)
            for m in range(3):
                g.wait_ge(s_sd[m], 16 * len([1 for gr in range(NCH)
                                             if gr % 3 == m]))
            g.collective_compute(
                "AllGather", mybir.AluOpType.bypass,
                replica_groups=[list(range(cfg.ncores))],
                ins=[tbl_loc[:, :]], outs=[tbl[:, :]],
            ).then_inc(s_cc)
            g.wait_ge(s_cc, 1)
            for c in range(NCH):
                if c >= 6:
                    g.wait_ge(s_mm, c - 5)
                for m in range(2):
                    g.dma_gather(
                        out_ap=gtile[c % 6][:, 8 * m:8 * m + 8, :],
                        in_ap=tbl[:, :],
                        idxs_ap=idx_sb[:, c * 128 + 64 * m:c * 128 + 64 * (m + 1)],
                        num_idxs=1024,
                        num_idxs_reg=1024,
                        elem_size=RB,
                        single_packet=False,
                        queue_num=c % 4,
                    ).then_inc(s_g[c % 8], 16)

        @block.tensor
        def _(pe):
            pe.wait_ge(s_in, 48)
            pe.wait_ge(s_in2, 48)
            pe.wait_ge(s_dvi, 1)
            pe.matmul(ps_A[0][:, 0:O_], ones4[:, :], b_sb[:, :],
                      start=True, stop=True).then_inc(s_bini, 1)
            for gr in range(NCH):
                glo = gr * 128
                if gr >= 2:
                    pe.wait_ge(s_w, gr - 1)       # ps_a[gr%2] free (exp read)
                pe.matmul(ps_a[gr % 2][:, 0:H_], xT_sb[:, 0, glo:glo + 128],
                          aw_sb[:, 0, :], start=True, stop=False)
                pe.matmul(ps_a[gr % 2][:, 0:H_], xT_sb[:, 1, glo:glo + 128],
                          aw_sb[:, 1, :], start=False, stop=True
                          ).then_inc(s_attn, 1)
                if gr >= 2:
                    pe.wait_ge(s_sm, gr - 1)      # ps_t[gr%2] free (DVE read)
                pe.matmul(ps_t[gr % 2][:, 0:SW], xT_sb[:, 0, glo:glo + 128],
                          W_sb[:, 0, :], start=True, stop=False)
                pe.matmul(ps_t[gr % 2][:, 0:SW], xT_sb[:, 1, glo:glo + 128],
                          W_sb[:, 1, :], start=False, stop=True
                          ).then_inc(s_tmm, 1)
            pe.wait_ge(s_bcp, 1)                  # binit copied out of ps_A[0]
            for c in range(NCH):
                pe.wait_ge(s_g[c % 8], 32 * (c // 8 + 1))
                if c >= 2:
                    pe.wait_ge(s_ep, c - 1)       # ps_A/ps_den[c%2] free (DVE)
                for gi in range(16):
                    pe.matmul(ps_A[c % 2][:, 0:SW],
                              blk_sb[:, 120 - 8 * gi:248 - 8 * gi],
                              gtile[c % 6][:, gi, 0:SW],
                              start=(gi == 0), stop=(gi == 15))
                for gi in range(16):
                    e = pe.matmul(ps_den[c % 2][:, 0:H_],
                                  blk_sb[:, 120 - 8 * gi:248 - 8 * gi],
                                  gtile[c % 6][:, gi, SW:SW + H_],
                                  start=(gi == 0), stop=(gi == 15))
                e.then_inc(s_mm, 1)

        @block.scalar
        def _(s):
            s.dma_start(out=aw_sb[:, :, :], in_=bass.AP(
                awp, 0, [[H_, 128], [128 * H_, DC], [1, H_]])).then_inc(s_in2, 16)
            s.dma_start(out=b_sb[:, :], in_=bass.AP(
                bp, 0, [[O_, H_], [1, O_]])).then_inc(s_in2, 16)
            s.dma_start(out=blk_sb[:, :], in_=blkp[:, :]).then_inc(s_in2, 16)
            for gr in range(NCH):
                s.wait_ge(s_attn, gr + 1)
                if gr >= 2:
                    s.wait_ge(s_sm, gr - 1)       # w4[gr%2] free (DVE read)
                s.activation(w4[gr % 2][:, :], ps_a[gr % 2][:, 0:H_],
                             mybir.ActivationFunctionType.Exp).then_inc(s_w, 1)
            for c in range(NCH):
                s.wait_ge(s_ep, c + 1)
                if c >= 2:
                    s.wait_ge(s_o[c % 2], 16 * (c // 2))  # ostage free
                s.activation(ostage[c % 2][:, :], acc_sb[c % 2][:, :],
                             mybir.ActivationFunctionType.Relu,
                             scale=1.0 / H_).then_inc(s_rel, 1)

        @block.vector
        def _(v):
            v.memset(ones4[:, :], 1.0)
            if cfg.VPF > VP:
                v.memset(xT_sb[:, :, VP:cfg.VPF], 0.0)
            for i in range(3):
                v.memset(stage[i][:, SW + H_:RB], 0.0)
            v.memset(zero_sb[:, :], 0.0).then_inc(s_dvi, 1)
            v.wait_ge(s_bini, 1)
            v.tensor_copy(binit_sb[:, :], ps_A[0][:, 0:O_]).then_inc(s_bcp, 1)
            for gr in range(NCH):
                v.wait_ge(s_tmm, gr + 1)          # ps_t ready
                v.wait_ge(s_w, gr + 1)            # w4 ready
                if gr >= 3:
                    v.wait_ge(s_sd[gr % 3], sd_wait(gr))  # stage free
                for h in range(H_):
                    # in1 is bypassed; xT_sb slice is just an initialized AP
                    v.scalar_tensor_tensor(
                        stage[gr % 3][:, h * O_:(h + 1) * O_],
                        ps_t[gr % 2][:, h * O_:(h + 1) * O_],
                        w4[gr % 2][:, h:h + 1], xT_sb[:, 0, 0:O_],
                        mybir.AluOpType.mult, mybir.AluOpType.bypass)
                v.tensor_copy(stage[gr % 3][:, SW:SW + H_],
                              w4[gr % 2][:, :]).then_inc(s_sm, 1)
            for c in range(NCH):
                v.wait_ge(s_mm, c + 1)
                if c >= 2:
                    v.wait_ge(s_rel, c - 1)       # acc_sb[c%2] free (ACT read)
                v.tensor_scalar_max(den_sb[c % 2][:, :], ps_den[c % 2][:, 0:H_],
                                    1e-30)
                v.drain()
                v.reciprocal(rcp_sb[c % 2][:, :], den_sb[c % 2][:, :])
                v.drain()
                v.scalar_tensor_tensor(
                    acc_sb[c % 2][:, :], ps_A[c % 2][:, 0:O_],
                    rcp_sb[c % 2][:, 0:1], binit_sb[:, :],
                    mybir.AluOpType.mult, mybir.AluOpType.add)
                for h in range(1, H_):
                    v.drain()
                    e = v.scalar_tensor_tensor(
                        acc_sb[c % 2][:, :], ps_A[c % 2][:, h * O_:(h + 1) * O_],
                        rcp_sb[c % 2][:, h:h + 1], acc_sb[c % 2][:, :],
                        mybir.AluOpType.mult, mybir.AluOpType.add)
                e.then_inc(s_ep, 1)

        @block.sync
        def _(sy):
            sy.dma_start(out=idx_sb[:, :], in_=idxp[:, :]).then_inc(s_in, 16)
            sy.dma_start(out=xT_sb[:, :, 0:VP], in_=bass.AP(
                xT, 0, [[VP, 128], [128 * VP, DC], [1, VP]])).then_inc(s_in, 16)
            sy.dma_start(out=W_sb[:, :, :], in_=bass.AP(
                W4p, 0, [[SW, 128], [128 * SW, DC], [1, SW]])).then_inc(s_in, 16)
            for gr in range(NCH):
                sy.wait_ge(s_sm, gr + 1)
                lo = gr * 128
                rows = min(128, VP - lo)
                sy.dma_start(
                    out=bass.AP(tbl_loc, lo * RB, [[RB, rows], [1, RB]]),
                    in_=stage[gr % 3][0:rows, :]).then_inc(s_sd[gr % 3], 16)
            for c in range(NCH):
                sy.wait_ge(s_rel, c + 1)
                lo = c * 128
                rows = min(128, VP - lo)
                sy.dma_start(out=bass.AP(out_ext, lo * O_, [[O_, rows], [1, O_]]),
                             in_=ostage[c % 2][0:rows, :]).then_inc(s_o[c % 2], 16)
            sy.wait_ge(s_o[0], 16 * ((NCH + 1) // 2))
            if NCH > 1:
                sy.wait_ge(s_o[1], 16 * (NCH // 2))

    sctx.close()
    ctx.close()
    nc.compile()
    return nc


def prep_core_inputs(cfg: Cfg, x, W, a, b, adj_lst, r):
    """Host-side shard/layout prep for core r (+ tiny param fold aw = W@a)."""
    VP, VPAD, NCH = cfg.VP, cfg.VPAD, cfg.NCH
    xs = np.ascontiguousarray(x[r * VP:(r + 1) * VP].T).astype(BF16NP)
    W = np.asarray(W, np.float32)
    W4 = np.ascontiguousarray(
        W.transpose(1, 0, 2).reshape(cfg.D, cfg.H * cfg.O)).astype(BF16NP)
    aw = np.einsum("hdo,ho->dh", W.astype(np.float64),
                   np.asarray(a, np.float64)).astype(BF16NP)
    adj = adj_lst[r * VP:(r + 1) * VP]
    rows = np.where(adj == cfg.V, VP,
                    (adj // VP) * VPAD + (adj % VP)).astype(np.int32)
    rows_p = np.full((NCH * 128, cfg.K), VP, np.int32)
    rows_p[:VP] = rows
    # dma_gather: edge i = 16*n_local + k lands at partition i%128 =
    # (n%8)*16+k, slot i//128 = n//8; index i is read from idx[i%16, i//16],
    # wrapped in 16 partitions and replicated across the 8 Q7 subcores.
    idx = np.tile(rows_p.T.astype(np.int16), (8, 1))
    blk = np.zeros((128, 248), np.float32)
    blk[np.arange(128), 120 + np.arange(128) // 16] = 1.0
    return {
        "xT": xs, "W4": W4, "aw": aw,
        "b": np.asarray(b, np.float32).astype(BF16NP),
        "blk": blk.astype(BF16NP), "idx": idx,
    }


_GRAPH_CACHE = {}


def kernel(x, W, a, b, adj_lst, mask_index, _cfg=None, _trace=False):
    cfg = _cfg or Cfg()
    x = np.asarray(x)
    adj_lst = np.asarray(adj_lst)
    assert int(mask_index) == cfg.V
    key = (cfg.V, cfg.ncores)
    if key not in _GRAPH_CACHE:
        _GRAPH_CACHE[key] = build_graph(cfg)
    nc = _GRAPH_CACHE[key]
    in_maps = [prep_core_inputs(cfg, x, W, a, b, adj_lst, r)
               for r in range(cfg.ncores)]
    res = run_bass_kernel_spmd(nc, in_maps, list(range(cfg.ncores)),
                               trace=_trace)
    out = np.concatenate([res.results[r]["out"] for r in range(cfg.ncores)], 0)
    kernel._last_exec_ns = res.exec_time_ns
    return out
